# revision 23
# baseline (speedup 1.0000x reference)
"""DGMG loss kernel for Trainium2, 8-core data-parallel over graphs.

Contract: kernel(**inputs) takes the FULL unsharded inputs (as in
reference.setup_inputs()) and returns the FULL output (scalar f32 loss).

Strategy:
- B=256 graphs of N=128 nodes -> 32 graphs per core. N == 128 partitions.
- Per-core state lives entirely in SBUF, feature-major: hVT[p, f, g, s].
- segment_sum over the (fixed) edge list becomes a per-graph dense
  128x128 adjacency matmul; the adjacency is built on host from src/dst.
- readout = colsum(hV_g) @ gpW + N*gpb  (segment_sum of a linear map).
- All PE matmuls run in bf16 (fp32 PSUM accumulation).
- The scatter-step readout is updated incrementally: hG += (hv-old)@gpW.
- log_softmax over the 2-class fan head folds to softplus(l1-l0).
- The initial readout hG0 is computed on host (it is a linear map of the
  input hV0) so the PE can start on step 0 as soon as the fan weights land.
- GCN transposes (feature-major -> node-major) run on the DMA X-bar
  (dma_start_transpose), not the PE.
- The per-graph constant part of the fs MLP is added by the Vector engine
  via a stride-0 broadcast AP for 3 of 4 output tiles; the 4th goes
  through the PE (rank-32 indicator matmul) for engine balance.
- Edge-loss ln(sigmoid(u)) terms are computed as -softplus(∓u) so the
  whole deferred loss tail needs only Softplus/Exp/Ln table loads.
"""
import sys
from contextlib import ExitStack

sys.path.insert(0, "/opt/trn_rl_repo")

import numpy as np
import ml_dtypes

import concourse.bacc as bacc
import concourse.tile as tile
import concourse.mybir as mybir
from concourse import bass_utils
from concourse.masks import make_identity

BF = mybir.dt.bfloat16
F32 = mybir.dt.float32
FP8 = mybir.dt.float8e4
AF = mybir.ActivationFunctionType
ALU = mybir.AluOpType
AX = mybir.AxisListType

B, N, D, G = 256, 128, 256, 512
S, T = 4, 2
NCORES = 8
GBL = B // NCORES          # 32 graphs per core
NF = D // 128              # 2 feature tiles
NG = G // 128              # 4 graph-hidden tiles
NCH = GBL * N // 512       # 8 chunks of 512 over (g, s)
EPS = 1e-7

_BUILT = None  # cached (nc, meta)

# packed-blob layouts (cols per piece), ordered by first use on device
WBF_LAYOUT = [
    ("wfan1", 4 * 4 * 128), ("wfan2d", 4),
    ("wfinit1", 4 * 4 * 128), ("wfinit2", 4 * 2 * 128), ("wgp", 2 * 4 * 128),
    ("wfae1", 6 * 6 * 128), ("wfae2", 6),
    ("wfs1a", 2 * 4 * 128), ("wfs1b", 2 * 4 * 128), ("wfs2", 4),
    ("wgcn", T * 2 * 2 * 128),
]
WBF_COLS = sum(c for _, c in WBF_LAYOUT)
# DMA split points (cols), grouping pieces by use phase
WBF_SPLITS = [2052, 6148, 10762, 12814, WBF_COLS]
BF32_LAYOUT = [
    ("bfan1", 4), ("bfinit1", 4), ("bfinit2", 2), ("bfae1", 6),
    ("bfs1", 4), ("bgpN", 4), ("bgcn", T * 2),
]
BF32_COLS = sum(c for _, c in BF32_LAYOUT)


# --------------------------------------------------------------------------
# device kernel builder
# --------------------------------------------------------------------------

def _declare_inputs(nc, zero_hv0, at_fp8):
    d = {}

    def di(name, shape, dt):
        d[name] = nc.dram_tensor(name, list(shape), dt, kind="ExternalInput")

    if not zero_hv0:
        di("hVT0", (128, NF * GBL * N), BF)
    di("AT", (128, GBL * N), FP8 if at_fp8 else BF)
    di("wbf", (128, WBF_COLS), BF)
    di("bf32", (128, BF32_COLS), F32)
    di("row32", (1, 2 * S * GBL + 4), F32)
    di("ind", (32, GBL * N), BF)
    di("selhot", (32, S * N), F32)
    di("hGT0", (128, NG * GBL), F32)
    return d


def _build(zero_hv0=True, at_fp8=True):
    nc = bacc.Bacc("TRN2", target_bir_lowering=False, debug=False)
    dins = _declare_inputs(nc, zero_hv0, at_fp8)
    dout = nc.dram_tensor("lossout", [1, 1], F32, kind="ExternalOutput")

    with tile.TileContext(nc) as tc, ExitStack() as stk:
        cp = stk.enter_context(tc.tile_pool(name="const", bufs=1))
        wp = stk.enter_context(tc.tile_pool(name="work", bufs=2))
        pp = stk.enter_context(tc.tile_pool(name="ps", bufs=2, space="PSUM"))

        # ---- persistent SBUF state ----
        hVT = cp.tile([128, NF, GBL, N], BF)            # node hidden, feature-major
        hGT = cp.tile([128, NG, GBL], F32)              # graph hidden, feature-major
        AT = cp.tile([128, GBL, N], FP8 if at_fp8 else BF)
        wbf = cp.tile([128, WBF_COLS], BF)
        bf32 = cp.tile([128, BF32_COLS], F32)
        row32 = cp.tile([1, 2 * S * GBL + 4], F32)
        ind = cp.tile([32, GBL * N], BF)
        selhot = cp.tile([32, S * N], F32)
        identity = cp.tile([128, 128], BF)
        rowacc = cp.tile([1, GBL], F32)
        colacc = cp.tile([GBL, 1], F32)
        ones32 = cp.tile([GBL, 1], F32)
        hGT_bf = cp.tile([128, NG, GBL], BF)
        draw_all = cp.tile([1, S * GBL], F32)
        pe_all = cp.tile([1, S * GBL], F32)             # pre-sigmoid fae logits u
        s32all = cp.tile([32, S * N], F32)
        h1all = cp.tile([128, 4, GBL * N], BF)

        # carve the packed blobs into named views
        _w = {}
        off = 0
        for nm, cols in WBF_LAYOUT:
            _w[nm] = wbf[:, off:off + cols]
            off += cols
        _b = {}
        off = 0
        for nm, cols in BF32_LAYOUT:
            _b[nm] = bf32[:, off:off + cols]
            off += cols
        wfan1 = _w["wfan1"].rearrange("p (a b c) -> p a b c", a=4, b=4, c=128)
        wfinit1 = _w["wfinit1"].rearrange("p (a b c) -> p a b c", a=4, b=4, c=128)
        wfinit2 = _w["wfinit2"].rearrange("p (a b c) -> p a b c", a=4, b=2, c=128)
        wfae1 = _w["wfae1"].rearrange("p (a b c) -> p a b c", a=6, b=6, c=128)
        wfs1a = _w["wfs1a"].rearrange("p (a b c) -> p a b c", a=2, b=4, c=128)
        wfs1b = _w["wfs1b"].rearrange("p (a b c) -> p a b c", a=2, b=4, c=128)
        wgp = _w["wgp"].rearrange("p (a b c) -> p a b c", a=2, b=4, c=128)
        wgcn = _w["wgcn"].rearrange("p (t a b c) -> p t a b c", t=T, a=2, b=2, c=128)
        wfan2d = _w["wfan2d"]
        wfae2 = _w["wfae2"]
        wfs2 = _w["wfs2"]
        bfan1 = _b["bfan1"]
        bfinit1 = _b["bfinit1"]
        bfinit2 = _b["bfinit2"]
        bfae1 = _b["bfae1"]
        bfs1 = _b["bfs1"]
        bgpN = _b["bgpN"]
        bgcn = _b["bgcn"].rearrange("p (t a) -> p t a", t=T, a=2)
        labn = row32[0:1, 0:S * GBL]
        labe = row32[0:1, S * GBL:2 * S * GBL]
        consts = row32[0:1, 2 * S * GBL:]

        # ---- loads: small/early blobs on the scalar ring, big on sync ----
        nc.scalar.dma_start(out=bf32[:], in_=dins["bf32"].ap())
        nc.scalar.dma_start(out=row32[:], in_=dins["row32"].ap())
        nc.scalar.dma_start(out=hGT[:].rearrange("p a b -> p (a b)"),
                            in_=dins["hGT0"].ap())
        w0 = 0
        for i, w1 in enumerate(WBF_SPLITS):
            nc.scalar.dma_start(out=wbf[:, w0:w1], in_=dins["wbf"].ap()[:, w0:w1])
            if i == 3:  # ind needed with the fs weights
                nc.scalar.dma_start(out=ind[:], in_=dins["ind"].ap())
            w0 = w1
        nc.scalar.dma_start(out=selhot[:], in_=dins["selhot"].ap())
        hVT_flat = hVT[:].rearrange("p a b c -> p (a b c)")
        if zero_hv0:
            HL = NF * GBL * N
            nc.vector.memset(hVT_flat[:, 0:HL // 2], 0.0)
            nc.gpsimd.memset(hVT_flat[:, HL // 2:], 0.0)
        else:
            Q = NF * GBL * N // 4
            for q in range(4):
                nc.sync.dma_start(out=hVT_flat[:, q * Q:(q + 1) * Q],
                                  in_=dins["hVT0"].ap()[:, q * Q:(q + 1) * Q])
        nc.sync.dma_start(out=AT[:].rearrange("p a b -> p (a b)"),
                          in_=dins["AT"].ap())

        make_identity(nc, identity[:])
        zeros1 = cp.tile([128, 1], F32)
        nc.vector.memset(zeros1[:], 0.0)
        zerobc = zeros1[:].to_broadcast([128, 512])
        nc.vector.memset(rowacc[:], 0.0)
        nc.vector.memset(colacc[:], 0.0)
        nc.vector.memset(ones32[:], 1.0)
        nc.vector.tensor_copy(out=hGT_bf[:], in_=hGT[:])

        # warm the PE clock gate while the weight DMAs land
        warmps = pp.tile([128, 128], F32, name="warmps", tag="b")
        for _ in range(40):
            nc.tensor.matmul(out=warmps[:], lhsT=identity[:], rhs=identity[:],
                             start=True, stop=True)

        # ---- helpers ----
        def readout_full(colsumT):
            """hGT <- colsum(hVT) @ gpW + N*gpb  (overwrites hGT, refreshes hGT_bf)"""
            colsum_bf = wp.tile([128, NF, GBL], BF, name="colsum_bf")
            nc.vector.tensor_copy(out=colsum_bf[:], in_=colsumT[:])
            hgps = pp.tile([128, NG, GBL], F32, name="hgps", tag="b")
            for ko in range(NG):
                for ki in range(NF):
                    nc.tensor.matmul(
                        out=hgps[:, ko, :], lhsT=wgp[:, ki, ko, :],
                        rhs=colsum_bf[:, ki, :],
                        start=(ki == 0), stop=(ki == NF - 1))
                nc.scalar.activation(
                    out=hGT[:, ko, :], in_=hgps[:, ko, :], func=AF.Identity,
                    bias=bgpN[:, ko:ko + 1])
                nc.scalar.activation(
                    out=hGT_bf[:, ko, :], in_=hgps[:, ko, :], func=AF.Identity,
                    bias=bgpN[:, ko:ko + 1])

        def mlp_to_psum(psum, win, bin_, rhs_tiles, nk, nko, act_out=None):
            """psum[:, ko, :] = sum_ki win[:,ki,ko,:] .T @ rhs_tiles(ki); then
            act_out[:, ko, :] = sigmoid(psum + bin_[:, ko])."""
            for ko in range(nko):
                for ki in range(nk):
                    nc.tensor.matmul(
                        out=psum[:, ko, :], lhsT=win[:, ki, ko, :],
                        rhs=rhs_tiles(ki), start=(ki == 0), stop=(ki == nk - 1))
                if act_out is not None:
                    nc.scalar.activation(
                        out=act_out[:, ko, :], in_=psum[:, ko, :],
                        func=AF.Sigmoid, bias=bin_[:, ko:ko + 1])

        # ---- generation steps ----
        for s in range(S):
            labn_s = labn[:, s * GBL:(s + 1) * GBL]
            labe_s = labe[:, s * GBL:(s + 1) * GBL]
            sel_s = selhot[:, s * N:(s + 1) * N]

            # ---------- fan: decide_add_node + loss1 ----------
            fanps = pp.tile([128, 4, GBL], F32, name="fanps", tag="b")
            h1fan = wp.tile([128, 4, GBL], BF, name="h1fan")
            mlp_to_psum(fanps, wfan1, bfan1, lambda ki: hGT_bf[:, ki, :], 4, 4,
                        act_out=h1fan)
            dps = pp.tile([1, GBL], F32, name="dps", tag="b")
            for k in range(4):
                nc.tensor.matmul(out=dps[:], lhsT=wfan2d[:, k:k + 1],
                                 rhs=h1fan[:, k, :], start=(k == 0), stop=(k == 3))
            nc.scalar.activation(out=draw_all[:, s * GBL:(s + 1) * GBL],
                                 in_=dps[:], func=AF.Identity,
                                 bias=consts[:, 0:1])

            # ---------- finit -> hv ----------
            g1ps = pp.tile([128, 4, GBL], F32, name="g1ps", tag="b")
            g1T = wp.tile([128, 4, GBL], BF, name="g1T")
            mlp_to_psum(g1ps, wfinit1, bfinit1, lambda ki: hGT_bf[:, ki, :], 4, 4,
                        act_out=g1T)
            hvps = pp.tile([128, NF, GBL], F32, name="hvps", tag="b")
            hvT = wp.tile([128, NF, GBL], BF, name="hvT")
            for ko in range(NF):
                for ki in range(4):
                    nc.tensor.matmul(
                        out=hvps[:, ko, :], lhsT=wfinit2[:, ki, ko, :],
                        rhs=g1T[:, ki, :], start=(ki == 0), stop=(ki == 3))
                nc.scalar.activation(
                    out=hvT[:, ko, :], in_=hvps[:, ko, :], func=AF.Identity,
                    bias=bfinit2[:, ko:ko + 1])

            # ---------- scatter node s + incremental readout ----------
            diffbf = wp.tile([128, NF, GBL], BF, name="diffbf")
            nc.vector.tensor_sub(out=diffbf[:], in0=hvT[:], in1=hVT[:, :, :, s])
            nc.vector.tensor_copy(out=hVT[:, :, :, s], in_=hvT[:])
            dhg = pp.tile([128, NG, GBL], F32, name="dhg", tag="b")
            for ko in range(NG):
                for ki in range(NF):
                    nc.tensor.matmul(
                        out=dhg[:, ko, :], lhsT=wgp[:, ki, ko, :],
                        rhs=diffbf[:, ki, :], start=(ki == 0), stop=(ki == NF - 1))
            nc.vector.tensor_add(out=hGT[:], in0=hGT[:], in1=dhg[:])
            nc.vector.tensor_copy(out=hGT_bf[:], in_=hGT[:])

            # ---------- fae: decide_add_edge + loss2 (keeps pre-sigmoid u) ----------
            ups = pp.tile([128, 6, GBL], F32, name="ups", tag="b")
            u1T = wp.tile([128, 6, GBL], BF, name="u1T")

            def fae_rhs(ki):
                return hGT_bf[:, ki, :] if ki < 4 else hvT[:, ki - 4, :]

            mlp_to_psum(ups, wfae1, bfae1, fae_rhs, 6, 6, act_out=u1T)
            peps = pp.tile([1, GBL], F32, name="peps", tag="b")
            for k in range(6):
                nc.tensor.matmul(out=peps[:], lhsT=wfae2[:, k:k + 1],
                                 rhs=u1T[:, k, :], start=(k == 0), stop=(k == 5))
            nc.scalar.activation(out=pe_all[:, s * GBL:(s + 1) * GBL],
                                 in_=peps[:], func=AF.Sigmoid,
                                 bias=consts[:, 1:2])

            # ---------- fs: select_node_to_add_edge ----------
            # cst[fo, g] = fs_w1b^T hv_g + fs_b1 (per-graph constant)
            cstps = pp.tile([128, 4, GBL], F32, name="cstps", tag="b")
            cst_fm = wp.tile([128, 3, GBL], F32, name="cst_fm")
            cst3_bf = wp.tile([128, GBL], BF, name="cst3_bf")
            for ko in range(4):
                for ki in range(NF):
                    nc.tensor.matmul(
                        out=cstps[:, ko, :], lhsT=wfs1b[:, ki, ko, :],
                        rhs=hvT[:, ki, :], start=(ki == 0), stop=(ki == NF - 1))
                if ko < 3:
                    nc.scalar.activation(
                        out=cst_fm[:, ko, :], in_=cstps[:, ko, :],
                        func=AF.Identity, bias=bfs1[:, ko:ko + 1])
                else:
                    nc.scalar.activation(
                        out=cst3_bf[:], in_=cstps[:, ko, :],
                        func=AF.Identity, bias=bfs1[:, ko:ko + 1])
            # ko=3 goes through the PE indicator path: transpose cst row 3
            ct3ps = pp.tile([32, 128], BF, name="ct3ps", tag="b")
            nc.tensor.transpose(out=ct3ps[:], in_=cst3_bf[:],
                                identity=identity[:])
            cstT3 = wp.tile([32, 128], BF, name="cstT3")
            nc.vector.tensor_copy(out=cstT3[:], in_=ct3ps[:])

            hVTf = [hVT[:, f, :, :].rearrange("p g t2 -> p (g t2)")
                    for f in range(NF)]
            for ch in range(NCH):
                cols = slice(ch * 512, (ch + 1) * 512)
                # ko 0-2 batched: one 3-bank psum, one vector add, one sigmoid
                zps3 = pp.tile([128, 3, 512], F32, name="zps3", tag="a")
                for ko in range(3):
                    for ki in range(NF):
                        nc.tensor.matmul(
                            out=zps3[:, ko, :], lhsT=wfs1a[:, ki, ko, :],
                            rhs=hVTf[ki][:, cols], start=(ki == 0),
                            stop=(ki == NF - 1))
                zsb3 = wp.tile([128, 3, 4, 128], F32, name="zsb3")
                nc.vector.tensor_add(
                    out=zsb3[:],
                    in0=zps3[:].rearrange("p k (g n) -> p k g n", g=4),
                    in1=cst_fm[:, :, 4 * ch:4 * ch + 4].to_broadcast(
                        [128, 3, 4, 128]))
                nc.scalar.activation(
                    out=h1all[:, 0:3, cols],
                    in_=zsb3[:].rearrange("p k g n -> p k (g n)"),
                    func=AF.Sigmoid)
                # ko 3 via the PE indicator path
                zps = pp.tile([128, 512], F32, name="zps", tag="b")
                for ki in range(NF):
                    nc.tensor.matmul(
                        out=zps[:], lhsT=wfs1a[:, ki, 3, :],
                        rhs=hVTf[ki][:, cols], start=(ki == 0), stop=False)
                nc.tensor.matmul(out=zps[:], lhsT=cstT3[:],
                                 rhs=ind[:, cols], start=False, stop=True)
                nc.scalar.activation(out=h1all[:, 3, cols], in_=zps[:],
                                     func=AF.Sigmoid)

            # ---------- gcn propagate: T layers (dead on the last step) ----------
            csT = wp.tile([128, NF, GBL], F32, name="csT")
            for t in range(T if s < S - 1 else 0):
                # node-major hV via PE transposes (identity matmul)
                hVn = wp.tile([128, NF, GBL, 128], BF, name="hVn", bufs=1)
                for f in range(NF):
                    for gb in range(GBL // 4):
                        tps = pp.tile([128, 4, 128], BF, name="tps", tag="b")
                        for j in range(4):
                            nc.tensor.transpose(
                                out=tps[:, j, :], in_=hVT[:, f, gb * 4 + j, :],
                                identity=identity[:])
                        if gb % 2 == 0:
                            nc.vector.tensor_copy(
                                out=hVn[:, f, gb * 4:gb * 4 + 4, :], in_=tps[:])
                        else:
                            nc.scalar.copy(
                                out=hVn[:, f, gb * 4:gb * 4 + 4, :], in_=tps[:])
                m1T = wp.tile([128, NF, GBL, 128], BF, name="m1T", bufs=1)
                for f in range(NF):
                    for gb in range(GBL // 4):
                        m1ps = pp.tile([128, 4, 128], F32, name="m1ps", tag="b",
                                       bufs=2)
                        for j in range(4):
                            g = gb * 4 + j
                            nc.tensor.matmul(
                                out=m1ps[:, j, :], lhsT=hVn[:, f, g, :],
                                rhs=AT[:, g, :], start=True, stop=True)
                        if (f * 8 + gb) % 2 == 0:
                            nc.vector.tensor_copy(
                                out=m1T[:, f, gb * 4:gb * 4 + 4, :], in_=m1ps[:])
                        else:
                            nc.scalar.copy(
                                out=m1T[:, f, gb * 4:gb * 4 + 4, :], in_=m1ps[:])
                m1Tf = [m1T[:, f, :, :].rearrange("p g t2 -> p (g t2)")
                        for f in range(NF)]
                for ko in range(NF):
                    for ch in range(NCH):
                        cols = slice(ch * 512, (ch + 1) * 512)
                        mm2ps = pp.tile([128, 512], F32, name="mm2ps", tag="a")
                        for ki in range(NF):
                            nc.tensor.matmul(
                                out=mm2ps[:], lhsT=wgcn[:, t, ki, ko, :],
                                rhs=m1Tf[ki][:, cols],
                                start=(ki == 0), stop=(ki == NF - 1))
                        if t < T - 1:
                            out_ap = hVT[:, ko, :, :].rearrange(
                                "p g t2 -> p (g t2)")[:, cols]
                            if ko == 0:
                                nc.scalar.activation(
                                    out=out_ap, in_=mm2ps[:], func=AF.Relu,
                                    bias=bgcn[:, t, ko:ko + 1])
                            else:
                                nc.vector.scalar_tensor_tensor(
                                    out=out_ap, in0=mm2ps[:],
                                    scalar=bgcn[:, t, ko:ko + 1], in1=zerobc,
                                    op0=ALU.add, op1=ALU.max)
                        else:
                            # last layer: per-graph evac; the accumulator
                            # yields the post-relu colsum (readout input) free
                            for j in range(4):
                                g = ch * 4 + j
                                gc = slice(j * 128, (j + 1) * 128)
                                if ko == 0:
                                    nc.scalar.activation(
                                        out=hVT[:, ko, g, :], in_=mm2ps[:, gc],
                                        func=AF.Relu, bias=bgcn[:, t, ko:ko + 1],
                                        accum_out=csT[:, ko, g:g + 1])
                                else:
                                    nc.vector.scalar_tensor_tensor(
                                        out=hVT[:, ko, g, :], in0=mm2ps[:, gc],
                                        scalar=bgcn[:, t, ko:ko + 1],
                                        in1=zerobc[:, 0:128],
                                        op0=ALU.add, op1=ALU.max,
                                        accum_out=csT[:, ko, g:g + 1])

            # ---------- update graph repr (scores MMs fill the PE gap) ----------
            if s < S - 1:
                readout_full(csT)
            scrow = wp.tile([1, GBL * N], F32, name="scrow")
            for ch in range(NCH):
                cols = slice(ch * 512, (ch + 1) * 512)
                scps = pp.tile([1, 512], F32, name="scps", tag="b")
                for ko in range(4):
                    nc.tensor.matmul(out=scps[:], lhsT=wfs2[:, ko:ko + 1],
                                     rhs=h1all[:, ko, cols], start=(ko == 0),
                                     stop=(ko == 3))
                nc.vector.tensor_copy(out=scrow[:, cols], in_=scps[:])
            nc.sync.dma_start(out=s32all[:, s * N:(s + 1) * N], in_=scrow[:])

        # ---- deferred loss math (one Exp phase, then one Ln phase) ----
        # Gate the scalar Exp/Ln ops on the last fs sigmoid output so the
        # list scheduler cannot hoist them (and their table loads) into the
        # middle of the step-3 sigmoid stream.
        gate = h1all[0:1, :, GBL * N - 32:]                      # [1, 4, 32]
        draw2 = wp.tile([1, S * GBL], F32, name="draw2")
        pe2 = wp.tile([1, S * GBL], F32, name="pe2")
        nc.vector.scalar_tensor_tensor(
            out=draw2[:].rearrange("p (a b) -> p a b", a=4),
            in0=draw_all[:].rearrange("p (a b) -> p a b", a=4), scalar=0.0,
            in1=gate, op0=ALU.add, op1=ALU.bypass)
        nc.vector.scalar_tensor_tensor(
            out=pe2[:].rearrange("p (a b) -> p a b", a=4),
            in0=pe_all[:].rearrange("p (a b) -> p a b", a=4), scalar=0.0,
            in1=gate, op0=ALU.add, op1=ALU.bypass)
        # Exp phase: loss1 softplus input + loss3 per-step exps
        expd = wp.tile([1, S * GBL], F32, name="expd")
        nc.scalar.activation(out=expd[:], in_=draw2[:], func=AF.Exp)
        s32v = s32all[:].rearrange("p (s n) -> p s n", s=S)
        mxa = wp.tile([32, S], F32, name="mxa")
        nc.vector.tensor_reduce(out=mxa[:], in_=s32v, axis=AX.X, op=ALU.max)
        suma = wp.tile([32, S], F32, name="suma")
        e32 = wp.tile([32, N], F32, name="e32")
        negmx = wp.tile([32, 1], F32, name="negmx")
        for st in range(S):
            nc.vector.tensor_scalar_mul(negmx[:], mxa[:, st:st + 1], -1.0)
            nc.scalar.activation(out=e32[:], in_=s32v[:, st, :], func=AF.Exp,
                                 bias=negmx[:], accum_out=suma[:, st:st + 1])
        # Ln phase: loss1 softplus, loss2 log-probs, loss3 log-sum
        spall = wp.tile([1, S * GBL], F32, name="spall")
        nc.scalar.activation(out=spall[:], in_=expd[:], func=AF.Ln, bias=1.0)
        t1a = wp.tile([1, S * GBL], F32, name="t1a")
        t2a = wp.tile([1, S * GBL], F32, name="t2a")
        nc.scalar.activation(out=t1a[:], in_=pe2[:], func=AF.Ln,
                             bias=consts[:, 2:3])
        nc.scalar.activation(out=t2a[:], in_=pe2[:], func=AF.Ln,
                             scale=-1.0, bias=consts[:, 3:4])
        lsuma = wp.tile([32, S], F32, name="lsuma")
        nc.scalar.activation(out=lsuma[:], in_=suma[:], func=AF.Ln)
        # vector combine
        l1b = wp.tile([1, S * GBL], F32, name="l1b")
        nc.vector.tensor_mul(out=l1b[:], in0=draw_all[:], in1=labn[:])
        nc.vector.tensor_sub(out=l1b[:], in0=spall[:], in1=l1b[:])
        d12 = wp.tile([1, S * GBL], F32, name="d12")
        nc.vector.tensor_sub(out=d12[:], in0=t1a[:], in1=t2a[:])
        nc.vector.tensor_mul(out=d12[:], in0=d12[:], in1=labe[:])
        nc.vector.tensor_add(out=d12[:], in0=d12[:], in1=t2a[:])
        nc.vector.tensor_sub(out=l1b[:], in0=l1b[:], in1=d12[:])
        for st in range(S):
            nc.vector.tensor_add(
                out=rowacc[:], in0=rowacc[:],
                in1=l1b[:].rearrange("p (s g) -> p s g", s=S)[:, st, :])
        pall = wp.tile([32, S * N], F32, name="pall")
        nc.vector.tensor_mul(out=pall[:], in0=s32all[:], in1=selhot[:])
        picked = wp.tile([32, S], F32, name="picked")
        nc.vector.tensor_reduce(
            out=picked[:], in_=pall[:].rearrange("p (s n) -> p s n", s=S),
            axis=AX.X, op=ALU.add)
        l3 = wp.tile([32, S], F32, name="l3")
        nc.vector.tensor_add(out=l3[:], in0=mxa[:], in1=lsuma[:])
        nc.vector.tensor_sub(out=l3[:], in0=l3[:], in1=picked[:])
        nc.vector.tensor_reduce(out=colacc[:], in_=l3[:], axis=AX.X, op=ALU.add)

        # ---- finalize: loss = sum(rowacc) + sum(colacc), to DRAM ----
        r1 = cp.tile([1, 1], F32)
        nc.vector.tensor_reduce(out=r1[:], in_=rowacc[:], axis=AX.X, op=ALU.add)
        cps = pp.tile([1, 1], F32, name="cps", tag="b")
        nc.tensor.matmul(out=cps[:], lhsT=colacc[:], rhs=ones32[:],
                         start=True, stop=True)
        losssb = cp.tile([1, 1], F32)
        nc.vector.tensor_add(out=losssb[:], in0=r1[:], in1=cps[:])
        nc.sync.dma_start(out=dout.ap(), in_=losssb[:])

    nc.compile()
    return nc


# --------------------------------------------------------------------------
# host-side input preparation
# --------------------------------------------------------------------------

def _bf(x):
    return np.ascontiguousarray(x).astype(ml_dtypes.bfloat16)


def _f32(x):
    return np.ascontiguousarray(x, dtype=np.float32)


def _tile_w(w, nki, nko):
    """[K, M] -> [128, nki, nko, 128] (lhsT tiles)."""
    K, M = w.shape
    assert K == nki * 128 and M == nko * 128
    return np.ascontiguousarray(
        w.reshape(nki, 128, nko, 128).transpose(1, 0, 2, 3).reshape(128, -1))


def _tile_b(b, n):
    return np.ascontiguousarray(b.reshape(n, 128).T)


def _variant_flags(inputs):
    inp = {k: np.asarray(v) for k, v in inputs.items()}
    zero_hv0 = not inp["hV0"].any()
    # adjacency entries are small edge counts; fp8e4m3 is exact for them
    counts = np.bincount(inp["src"].astype(np.int64) * N
                         + (inp["dst"].astype(np.int64) % N),
                         minlength=B * N * N)
    at_fp8 = counts.max() <= 15
    return zero_hv0, at_fp8


def _prep_inputs(inputs, zero_hv0, at_fp8):
    inp = {k: np.asarray(v) for k, v in inputs.items()}
    f32 = np.float32

    # adjacency blocks AT[g, s, d]
    src = inp["src"].astype(np.int64)
    dst = inp["dst"].astype(np.int64)
    flat = np.bincount(src * N + (dst % N), minlength=B * N * N)
    ATh = flat.reshape(B, N, N).astype(f32)
    at_np = ml_dtypes.float8_e4m3fn if at_fp8 else ml_dtypes.bfloat16

    # weights (shared across cores), packed into blobs per WBF/BF32 layouts
    pieces = {
        "wfan1": _bf(_tile_w(inp["fan_w1"], 4, 4)),
        "wfinit1": _bf(_tile_w(inp["finit_w1"], 4, 4)),
        "wfinit2": _bf(_tile_w(inp["finit_w2"], 4, 2)),
        "wfae1": _bf(_tile_w(inp["fae_w1"], 6, 6)),
        "wfs1a": _bf(_tile_w(inp["fs_w1"][:D], 2, 4)),
        "wfs1b": _bf(_tile_w(inp["fs_w1"][D:], 2, 4)),
        "wgp": _bf(_tile_w(inp["gpW"], 2, 4)),
        "wgcn": _bf(np.concatenate(
            [_tile_w(inp["gcn_W"][t], 2, 2) for t in range(T)], axis=1)),
        "wfan2d": _bf((inp["fan_w2"][:, 1] - inp["fan_w2"][:, 0]).reshape(4, 128).T),
        "wfae2": _bf(inp["fae_w2"].reshape(6, 128).T),
        "wfs2": _bf(inp["fs_w2"][:, 0].reshape(4, 128).T),
    }
    wbf = np.concatenate([pieces[nm] for nm, _ in WBF_LAYOUT], axis=1)
    bpieces = {
        "bfan1": _f32(_tile_b(inp["fan_b1"], 4)),
        "bfinit1": _f32(_tile_b(inp["finit_b1"], 4)),
        "bfinit2": _f32(_tile_b(inp["finit_b2"], 2)),
        "bfae1": _f32(_tile_b(inp["fae_b1"], 6)),
        "bfs1": _f32(_tile_b(inp["fs_b1"], 4)),
        "bgpN": _f32(_tile_b(N * inp["gpb"], 4)),
        "bgcn": _f32(np.stack(
            [inp["gcn_b"][t].reshape(2, 128).T for t in range(T)], axis=1
        ).reshape(128, T * 2)),
    }
    bf32 = np.concatenate([bpieces[nm] for nm, _ in BF32_LAYOUT], axis=1)
    shared = {
        "wbf": np.ascontiguousarray(wbf),
        "bf32": np.ascontiguousarray(bf32),
        "ind": _bf((np.arange(32)[:, None] == (np.arange(GBL * N) // N)[None, :])),
    }
    consts_row = np.array([inp["fan_b2"][1] - inp["fan_b2"][0], inp["fae_b2"][0],
                           EPS, 1.0 + EPS], dtype=f32)

    hV0 = inp["hV0"].astype(f32)
    # initial readout on host: hG0 = segment_sum(hV0 @ gpW) + N*gpb  (exact)
    colsum0 = hV0.reshape(B, N, D).sum(axis=1)
    hG0 = colsum0 @ inp["gpW"].astype(f32) + N * inp["gpb"].astype(f32)  # [B, G]
    labn = inp["labels_node"].astype(f32)   # [S, B]
    labe = inp["labels_edge"].astype(f32)
    sel = inp["node_select"]
    selhot = (np.arange(N)[None, None, :] == sel[:, :, None]).astype(f32)  # [S,B,N]

    in_maps = []
    for c in range(NCORES):
        gs = slice(c * GBL, (c + 1) * GBL)
        # hVT[p, f, g, s] = hV[(g*128+s), f*128+p]
        hvc = hV0.reshape(B, N, NF, 128)[gs]            # [GBL, s, f, p]
        hVT = np.ascontiguousarray(
            hvc.transpose(3, 2, 0, 1).reshape(128, -1))
        ATc = np.ascontiguousarray(
            ATh[gs].transpose(1, 0, 2).reshape(128, -1))  # [s(p), g, d]
        m = dict(shared)
        if not zero_hv0:
            m["hVT0"] = _bf(hVT)
        m["AT"] = np.ascontiguousarray(ATc).astype(at_np)
        m["row32"] = _f32(np.concatenate(
            [labn[:, gs].reshape(-1), labe[:, gs].reshape(-1), consts_row]
        ).reshape(1, -1))
        # selhot tile [32, S*N]: partition = graph-in-core
        m["selhot"] = _f32(
            selhot[:, gs].transpose(1, 0, 2).reshape(GBL, S * N))
        # hGT0[p, ko, g] = hG0[g, ko*128 + p]
        m["hGT0"] = _f32(
            hG0[gs].reshape(GBL, NG, 128).transpose(2, 1, 0).reshape(128, -1))
        in_maps.append(m)
    return in_maps


# --------------------------------------------------------------------------
# public entry
# --------------------------------------------------------------------------

_VARIANTS = {}


def kernel(**inputs) -> np.ndarray:
    flags = _variant_flags(inputs)
    if flags not in _VARIANTS:
        _VARIANTS[flags] = _build(*flags)
    nc = _VARIANTS[flags]
    in_maps = _prep_inputs(inputs, *flags)
    res = bass_utils.run_bass_kernel_spmd(
        nc, in_maps, core_ids=list(range(NCORES)))
    total = np.float32(0.0)
    for r in res.results:
        total += r["lossout"].reshape(())
    return np.float32(total / B)
